# revision 18
# baseline (speedup 1.0000x reference)
"""Bahdanau-style attention kernel for Trainium2, data-parallel over batch on
8 NeuronCores.

Reference computation (per batch b):
    enc   = enc_state @ W_enc.T            # [S, H]
    dec   = W_dec @ dec_state              # [H]
    t     = tanh(enc + dec)                # [S, H]
    en    = t @ W_energy.T                 # [S]
    en    = where(arange(S) < L, en, -inf)
    alpha = softmax(en)                    # [S]
    ctx   = alpha @ enc_state              # [2H]

Device mapping (per core, B_LOC=4 batches):
  - enc is streamed once per batch as four contiguous 2MB bf16 tiles in
    [e-part, s-free] layout (prepacked host-side), kept resident in SBUF for
    the whole batch so the context needs no second load.
  - Projection: bf16 matmuls with W_encT tiles stationary, fp32 PSUM accum.
    ScalarE applies tanh fused with the +dec bias (host-precomputed, fp32).
  - Energy reduction over h: M=1 matmul chain over the 8 h-tiles.
  - Masking: additive -1e30 mask (host-precomputed from src_length) fused
    into the PSUM->SBUF eviction add on VectorE.
  - Softmax on the [1, S] row: reduce_max, exp with accumulated sum,
    reciprocal, scale.
  - Context: normalized alphas row is broadcast to 128 partitions with a
    rank-1 PE matmul per s-block, then VectorE tensor_tensor_reduce fuses
    (te * alpha) with the free-dim sum into per-(e-tile) partials, chained
    across s-blocks via the reduce init scalar.
"""

import numpy as np
import ml_dtypes

import concourse.tile as tile
from concourse import bacc, mybir
from concourse.bass_utils import run_bass_kernel_spmd

B, S, H = 32, 2048, 1024
E = 2 * H
NCORES = 8
B_LOC = B // NCORES
P = 128
SBLK = 512
NSB = S // SBLK     # s-blocks per batch
ET = E // P         # e-tiles (contraction tiles of the projection)
HT = H // P         # h-tiles
NEG = np.float32(-1e30)

f32 = mybir.dt.float32
f32r = mybir.dt.float32r
bf16 = mybir.dt.bfloat16
AF = mybir.ActivationFunctionType
ALU = mybir.AluOpType


def _finish_block(nc, sb, b, erow, amrow, pet, bms, small, mybir_mod):
    """Evict the energy PSUM row for s-block sb (masked add into erow) and
    compute its block max (online, so the final max is a 3-op tree)."""
    nc.vector.tensor_add(
        erow[0:1, sb * SBLK : (sb + 1) * SBLK],
        pet[:],
        amrow[0:1, sb * SBLK : (sb + 1) * SBLK],
    )
    bm = small.tile([1, 1], f32, tag=f"bm{sb}", name=f"bm_{b}_{sb}")
    nc.vector.reduce_max(
        bm[:],
        erow[0:1, sb * SBLK : (sb + 1) * SBLK],
        axis=mybir_mod.AxisListType.X,
    )
    bms.append(bm)


def build_program(b_loc=B_LOC, n_cores=NCORES, n_iter=1):
    nc = bacc.Bacc(
        "TRN2", target_bir_lowering=False, debug=False, num_devices=n_cores
    )
    # encp[b, sb, p, et, j] = enc_state[b, sb*SBLK + j, et*P + p]
    encp_d = nc.dram_tensor("encp", [b_loc, NSB, P, ET, SBLK], bf16,
                            kind="ExternalInput")
    # wenc[p, et, h] = W_enc[h, et*P + p]
    wenc_d = nc.dram_tensor("wenc", [P, ET, H], bf16, kind="ExternalInput")
    # wem[p, ht] = W_energy[0, ht*P + p]
    wem_d = nc.dram_tensor("wem", [P, HT], bf16, kind="ExternalInput")
    # decp[p, ht*b_loc + b] = (W_dec @ dec_state[b])[ht*P + p]
    decp_d = nc.dram_tensor("decp", [P, HT * b_loc], f32, kind="ExternalInput")
    amask_d = nc.dram_tensor("amask", [1, b_loc * S], f32, kind="ExternalInput")
    # ctx[b, p, et] -> host transposes to [b, e]
    ctx_d = nc.dram_tensor("ctx", [b_loc, P, ET], f32, kind="ExternalOutput")
    alph_d = nc.dram_tensor("alph", [b_loc, S], f32, kind="ExternalOutput")

    with tile.TileContext(nc) as tc:
        with tc.tile_pool(name="persist", bufs=1) as persist:
            wenc_sb = persist.tile([P, ET, H], bf16)
            nc.sync.dma_start(out=wenc_sb[:], in_=wenc_d[:])
            wem_sb = persist.tile([P, HT], bf16)
            nc.sync.dma_start(out=wem_sb[:], in_=wem_d[:])
            decp_sb = persist.tile([P, HT * b_loc], f32)
            nc.sync.dma_start(out=decp_sb[:], in_=decp_d[:])
            ones_sb = persist.tile([1, P], f32)
            nc.vector.memset(ones_sb, 1.0)

            with (
                tc.tile_pool(name="te", bufs=1) as te_pool,
                tc.tile_pool(name="tt", bufs=2) as tt_pool,
                tc.tile_pool(name="rows", bufs=2) as rows,
                tc.tile_pool(name="small", bufs=2) as small,
                tc.tile_pool(name="acc", bufs=2) as accp,
                tc.tile_pool(name="pp", bufs=2, space="PSUM") as pp,
                tc.tile_pool(name="pe", bufs=2, space="PSUM") as pe,
                tc.tile_pool(name="pb", bufs=2, space="PSUM") as pb,
            ):
                def emit_ctx(st):
                    # Context phase for a FINISHED batch, emitted inside the
                    # next batch's matmul stream so its PE ops (bc broadcast)
                    # never leave the PE waiting on the softmax chain.
                    b0, tes0, arow0, rz0 = st
                    parts = []
                    for sb in range(NSB):
                        pbc = pb.tile([P, SBLK], f32, tag="pbc")
                        nc.tensor.matmul(
                            pbc[:],
                            ones_sb[:],
                            arow0[0:1, sb * SBLK : (sb + 1) * SBLK],
                            start=True,
                            stop=True,
                        )
                        bc16 = small.tile([P, SBLK], bf16, tag="bc16")
                        nc.vector.tensor_copy(bc16[:], pbc[:])
                        part = accp.tile([P, ET], f32, tag="part", bufs=8)
                        dump = small.tile([P, SBLK], bf16, tag="dump")
                        for et in range(ET):
                            nc.vector.scalar_tensor_tensor(
                                out=dump[:],
                                in0=tes0[sb][:, et, :],
                                scalar=1.0,
                                in1=bc16[:],
                                op0=ALU.mult,
                                op1=ALU.mult,
                                accum_out=part[:, et : et + 1],
                            )
                        parts.append(part)
                    # alphas output (normalized) — off the bc critical path
                    aout = rows.tile([1, S], f32, tag="aout", bufs=1)
                    nc.vector.tensor_scalar_mul(aout[:], arow0[:], rz0[0:1, 0:1])
                    nc.sync.dma_start(out=alph_d[b0], in_=aout[:])
                    s01 = accp.tile([P, ET], f32, tag="s01")
                    nc.vector.tensor_add(s01[:], parts[0][:], parts[1][:])
                    s23 = accp.tile([P, ET], f32, tag="s23")
                    nc.vector.tensor_add(s23[:], parts[2][:], parts[3][:])
                    ctxu = accp.tile([P, ET], f32, tag="ctxu")
                    nc.vector.tensor_add(ctxu[:], s01[:], s23[:])
                    # normalize: broadcast 1/Z to all partitions with a tiny
                    # rank-1 matmul, then a per-partition scale on ScalarE
                    prz = pb.tile([P, 1], f32, tag="prz", bufs=1)
                    nc.tensor.matmul(
                        prz[:], ones_sb[:], rz0[0:1, 0:1], start=True, stop=True
                    )
                    rz128 = small.tile([P, 1], f32, tag="rz128")
                    nc.vector.tensor_copy(rz128[:], prz[:])
                    ctx_sb = accp.tile([P, ET], f32, tag="ctxsb")
                    nc.scalar.mul(ctx_sb[:], ctxu[:], rz128[:, 0:1])
                    nc.sync.dma_start(out=ctx_d[b0], in_=ctx_sb[:])

                pending = None
                for b in [bb for _ in range(n_iter) for bb in range(b_loc)]:
                    erow = rows.tile([1, S], f32, tag="erow")
                    amrow = rows.tile([1, S], f32, tag="amrow", bufs=2)
                    nc.sync.dma_start(
                        out=amrow[:], in_=amask_d[0:1, b * S : (b + 1) * S]
                    )
                    tes = []
                    bms = []
                    pend = []  # deferred (ht, tt, pet) energy matmuls
                    for sb in range(NSB):
                        te = te_pool.tile([P, ET, SBLK], bf16, tag="te", bufs=6)
                        nc.sync.dma_start(out=te[:], in_=encp_d[b, sb])
                        tes.append(te)
                        pet = pe.tile([1, SBLK], f32, tag="pet", bufs=3)
                        for ht in range(HT):
                            ppt = pp.tile([P, SBLK], f32, tag="ppt")
                            for et in range(ET):
                                nc.tensor.matmul(
                                    ppt[:],
                                    wenc_sb[:, et, ht * P : (ht + 1) * P],
                                    te[:, et, :],
                                    start=(et == 0),
                                    stop=(et == ET - 1),
                                )
                            tt = tt_pool.tile([P, SBLK], bf16, tag="tt", bufs=5)
                            nc.scalar.activation(
                                tt[:],
                                ppt[:],
                                AF.Tanh,
                                bias=decp_sb[:, ht * b_loc + b : ht * b_loc + b + 1],
                            )
                            # defer the energy matmul two chains so the PE
                            # never waits on the tanh eviction
                            pend.append((ht, tt, pet))
                            if len(pend) > 2:
                                h2, t2, p2 = pend.pop(0)
                                nc.tensor.matmul(
                                    p2[:],
                                    wem_sb[:, h2 : h2 + 1],
                                    t2[:],
                                    start=(h2 == 0),
                                    stop=(h2 == HT - 1),
                                )
                            if sb == 0 and ht == 3 and pending is not None:
                                emit_ctx(pending)
                                pending = None
                        if sb > 0:
                            _finish_block(nc, sb - 1, b, erow, amrow, pe_tiles,
                                          bms, small, mybir)
                        pe_tiles = pet
                    # drain deferred energy matmuls, then close the last block
                    for h2, t2, p2 in pend:
                        nc.tensor.matmul(
                            p2[:],
                            wem_sb[:, h2 : h2 + 1],
                            t2[:],
                            start=(h2 == 0),
                            stop=(h2 == HT - 1),
                        )
                    _finish_block(nc, NSB - 1, b, erow, amrow, pe_tiles, bms,
                                  small, mybir)
                    # combine the per-block maxes
                    m01 = small.tile([1, 1], f32, tag="m01")
                    nc.vector.tensor_max(m01[:], bms[0][:], bms[1][:])
                    m23 = small.tile([1, 1], f32, tag="m23")
                    nc.vector.tensor_max(m23[:], bms[2][:], bms[3][:])
                    mx = small.tile([1, 1], f32, tag="mx")
                    nc.vector.tensor_max(mx[:], m01[:], m23[:])
                    nmx = small.tile([1, 1], f32, tag="nmx")
                    nc.vector.tensor_scalar_mul(nmx[:], mx[:], -1.0)
                    zs = small.tile([1, 1], f32, tag="zs")
                    arow = rows.tile([1, S], f32, tag="arow")
                    nc.scalar.activation(
                        arow[:], erow[:], AF.Exp, bias=nmx[0:1, 0:1], accum_out=zs[:]
                    )
                    rz = small.tile([1, 1], f32, tag="rz")
                    nc.vector.reciprocal(rz[:], zs[:])
                    pending = (b, tes, arow, rz)
                emit_ctx(pending)
    nc.compile()
    return nc


_prog = None


def _get_prog():
    global _prog
    if _prog is None:
        _prog = build_program()
    return _prog


def _to_bf16(a):
    return np.asarray(a, dtype=np.float32).astype(ml_dtypes.bfloat16)


def _build_in_maps(dec_state, enc_state, src_length, W_enc, W_dec, W_energy):
    # wenc[p, et, h] = W_enc[h, et*P + p]
    wenc = np.ascontiguousarray(
        _to_bf16(W_enc).T.reshape(ET, P, H).transpose(1, 0, 2)
    )
    wem = np.ascontiguousarray(_to_bf16(W_energy)[0].reshape(HT, P).T)
    # dec projection on host: decall[h, B] = W_dec @ dec_state[:, 0, :].T
    decall = (W_dec.astype(np.float64) @ dec_state[:, 0, :].astype(np.float64).T)
    decall = decall.astype(np.float32)  # [H, B]
    iota = np.arange(S, dtype=np.int64)

    in_maps = []
    for c in range(NCORES):
        sl = slice(c * B_LOC, (c + 1) * B_LOC)
        lens = src_length[sl].astype(np.int64)
        amask = np.where(iota[None, :] < lens[:, None], np.float32(0.0), NEG).reshape(1, B_LOC * S)
        # encp[b, sb, p, et, j] = enc[b, sb*SBLK+j, et*P+p]
        encp = np.ascontiguousarray(
            _to_bf16(enc_state[sl])
            .reshape(B_LOC, NSB, SBLK, ET, P)
            .transpose(0, 1, 4, 3, 2)
        )
        # decp[p, ht*b_loc + b] = decall[ht*P + p, c*B_LOC + b]
        decp = np.ascontiguousarray(
            decall[:, sl].reshape(HT, P, B_LOC).transpose(1, 0, 2).reshape(P, HT * B_LOC)
        )
        in_maps.append(
            {
                "encp": encp,
                "wenc": wenc,
                "wem": wem,
                "decp": decp,
                "amask": amask.astype(np.float32),
            }
        )
    return in_maps


def _prepare_in_maps(inputs):
    return _build_in_maps(
        np.asarray(inputs["dec_state"], dtype=np.float32),
        np.asarray(inputs["enc_state"], dtype=np.float32),
        np.asarray(inputs["src_length"]),
        np.asarray(inputs["W_enc"], dtype=np.float32),
        np.asarray(inputs["W_dec"], dtype=np.float32),
        np.asarray(inputs["W_energy"], dtype=np.float32),
    )


def kernel(dec_state, enc_state, src_length, W_enc, W_dec, W_energy):
    in_maps = _build_in_maps(
        np.asarray(dec_state, dtype=np.float32),
        np.asarray(enc_state, dtype=np.float32),
        np.asarray(src_length),
        np.asarray(W_enc, dtype=np.float32),
        np.asarray(W_dec, dtype=np.float32),
        np.asarray(W_energy, dtype=np.float32),
    )
    nc = _get_prog()
    try:
        res = run_bass_kernel_spmd(nc, in_maps, list(range(NCORES)))
    except Exception:
        res = run_bass_kernel_spmd(nc, in_maps, list(range(NCORES)))
    # ctx comes back [b_loc, P, ET]; full e index = et*P + p
    ctx = np.concatenate(
        [np.asarray(r["ctx"]).transpose(0, 2, 1).reshape(B_LOC, E)
         for r in res.results], 0
    ).reshape(B, 1, E)
    alph = np.concatenate(
        [np.asarray(r["alph"]) for r in res.results], 0
    ).reshape(B, 1, S)
    return ctx, alph
